# revision 1
# baseline (speedup 1.0000x reference)
"""Trainium2 Bass kernel for the nn_MultiHeadAttention problem.

Data-parallel over batch: each of the 8 NeuronCores processes one batch
element independently (no collectives).

Mask compaction: the host gathers only the valid query/key positions
(QMask/KMask true), padded to a multiple of 128, and scatters the
output back (masked query rows are exactly zero in the reference).
With ~50% random masks this cuts the attention work ~4x.  The tile
counts (ntq, ntk) are chosen from the actual masks at kernel() time and
a bass program is compiled per shape, so any mask density works.  If
the max query count only slightly exceeds a 512 multiple, the device is
capped there (single 512-wide matmul chunks, no remainder tiles) and
the few overflow queries are computed exactly on the host at gather.

Per-core dataflow (E=1024, H=16, D=64; Lq=ntq*128 queries, Lk=ntk*128
keys after compaction; e-chunks of 128 = 2 heads):

  proj:  one blockdiag weight per chunk projects q and k in a single
         fused rhs ([QTc | KTc]), both heads at once; v2 per k-tile
         gets a validity "ones" column per head.
  scores: s[k,q] psum = kT_h_slice.T @ qT_h (bf16); P = exp(s/8), one
         ACT op per [128,Lq] tile -> bf16.  No max subtraction
         (|s|/8 <~ 13); masked/pad keys have v-rows and ones-column
         zeroed, reproducing masked_fill+softmax exactly.
  PV:    out[65,q] psum = sum_k v2_slice.T @ P_slice (bf16); row 64 is
         the softmax denominator.  Fast psum evacuation on DVE (denom
         row -> dstack via partition-64 staging + DMA shuffle, rows
         0:64 -> ct unnormalized); reciprocals in three batches off the
         critical path, then DRAM-bounce broadcast + one DVE multiply
         per head normalizes ct in place.
  final: split output projection after the main loop: part A (chunks
         0-6, already normalized) overlaps the last normalize chain;
         part B adds chunk 7 via ysum (SBUF f32) and writes Y.
"""

import math
import os
import sys

import numpy as np

try:
    import concourse  # noqa: F401
except ImportError:  # pragma: no cover
    for _p in ("/opt/trn_rl_repo", os.path.expanduser("~/.axon_site/_ro/trn_rl_repo")):
        if os.path.isdir(_p) and _p not in sys.path:
            sys.path.insert(0, _p)

import ml_dtypes

import concourse.bass as bass
import concourse.tile as tile
from concourse import bacc, mybir

B, L, E, H, D = 8, 1024, 1024, 16, 64
P = 128          # partitions
NCH = E // P     # 8 e-chunks (2 heads each)
F32 = mybir.dt.float32
BF16 = mybir.dt.bfloat16

# normalize batches: (head range start, end, after-chunk)
NORM_BATCHES = [(0, 8, 3), (8, 14, 6), (14, 16, 7)]


def _chunks(n, step=512):
    return [(s, min(s + step, n)) for s in range(0, n, step)]


def build_bass(ntq, ntk):
    Lq, Lk = ntq * P, ntk * P
    nc = bacc.Bacc(None, target_bir_lowering=False, debug=False)

    QT = nc.declare_dram_parameter("QT", [E, Lq], BF16, isOutput=False)
    KT = nc.declare_dram_parameter("KT", [E, Lk], BF16, isOutput=False)
    VT = nc.declare_dram_parameter("VT", [E, Lk], BF16, isOutput=False)
    W2 = nc.declare_dram_parameter("W2", [P, NCH, P], BF16, isOutput=False)
    OB = nc.declare_dram_parameter("OB", [E, E], BF16, isOutput=False)
    KM = nc.declare_dram_parameter("KM", [P, ntk], F32, isOutput=False)
    Y = nc.declare_dram_parameter("Y", [Lq, E], F32, isOutput=True)
    rbounce = nc.dram_tensor("rbounce", [H, Lq], BF16)

    deep = Lq <= 512  # single-bank score tiles -> deeper PSUM pipelining
    with tile.TileContext(nc) as tc:
        with (
            tc.tile_pool(name="singles", bufs=1) as singles,
            tc.tile_pool(name="qkT", bufs=2) as qkT,
            tc.tile_pool(name="vaug", bufs=2) as vaug,
            tc.tile_pool(name="ppool", bufs=2) as ppool,
            tc.tile_pool(name="ystage", bufs=2) as ystage,
            tc.tile_pool(name="bcpool", bufs=3) as bcpool,
            tc.tile_pool(name="dtpool", bufs=2) as dtpool,
            tc.tile_pool(name="psbig", bufs=4 if deep else 2,
                         space="PSUM") as psbig,
            tc.tile_pool(name="pspv", bufs=2 if deep else 1,
                         space="PSUM") as pspv,
            tc.tile_pool(name="pssmall", bufs=2, space="PSUM") as pssmall,
        ):
            # --- persistent SBUF tensors -------------------------------
            qts = singles.tile([P, NCH, Lq], BF16)
            kts = singles.tile([P, NCH, Lk], BF16)
            vts = singles.tile([P, NCH, Lk], BF16)
            obs = singles.tile([P, NCH, E], BF16)
            w2s = singles.tile([P, NCH, P], BF16)
            kms = singles.tile([P, ntk], F32)
            ct = singles.tile([P, NCH, Lq], BF16)
            ysum = singles.tile([P, ntq, E], F32)
            dstacks = []
            rstacks = []
            for bi, (h0, h1, _) in enumerate(NORM_BATCHES):
                ds = singles.tile([(h1 - h0) * ntq, P], F32, tag=f"ds{bi}")
                rs = singles.tile([(h1 - h0) * ntq, P], BF16, tag=f"rs{bi}")
                dstacks.append(ds)
                rstacks.append(rs)

            # --- input DMAs (small/consts first, then per-chunk) -------
            nc.gpsimd.dma_start(out=w2s[:], in_=W2[:])
            nc.gpsimd.dma_start(out=kms[:], in_=KM[:])
            # PE warmup: ~8us of dummy matmuls while input DMAs land, so
            # the HAM clock gate opens before real work starts
            warm = singles.tile([P, 512], BF16)
            nc.vector.memset(warm[:], 0.0)
            for wi in range(16):
                wps = pssmall.tile([P, 512], F32, tag="small")
                nc.tensor.matmul(out=wps[:], lhsT=warm[:, 0:128], rhs=warm[:],
                                 start=True, stop=True)
            for c in range(NCH):
                nc.sync.dma_start(out=qts[:, c, :], in_=QT[c * P:(c + 1) * P, :])
                nc.sync.dma_start(out=kts[:, c, :], in_=KT[c * P:(c + 1) * P, :])
                nc.sync.dma_start(out=vts[:, c, :], in_=VT[c * P:(c + 1) * P, :])
            for c in range(NCH):
                nc.sync.dma_start(out=obs[:, c, :], in_=OB[c * P:(c + 1) * P, :])

            def normalize_batch(bi):
                h0, h1, _ = NORM_BATCHES[bi]
                with nc.allow_low_precision(reason="softmax recip bf16"):
                    nc.vector.reciprocal(out=rstacks[bi][:], in_=dstacks[bi][:])
                nc.gpsimd.dma_start(out=rbounce[h0:h1, :], in_=rstacks[bi][:])
                for h in range(h0, h1):
                    c, hf = h // 2, h % 2
                    bcs = bcpool.tile([P, Lq], BF16)
                    src = rbounce[h:h + 1, :]
                    bc_in = bass.AP(
                        tensor=src.tensor, offset=src.offset,
                        ap=[[0, P], list(src.ap[-1])])
                    nc.gpsimd.dma_start(out=bcs[:], in_=bc_in)
                    sl = ct[64 * hf:64 * hf + 64, c, :]
                    nc.vector.tensor_mul(sl, sl, bcs[64 * hf:64 * hf + 64, :])

            def final_mms(t, ytiles, crange):
                for c in crange:
                    for eh in range(2):
                        nc.tensor.matmul(
                            out=ytiles[eh][:],
                            lhsT=ct[:, c, t * P:(t + 1) * P],
                            rhs=obs[:, c, 512 * eh:512 * (eh + 1)],
                            start=(c == crange[0]), stop=(c == crange[-1]),
                        )

            # --- main loop over e-chunks (2 heads each) ----------------
            for c in range(NCH):
                # fused q/k projection for both heads of this chunk
                qkt2 = qkT.tile([P, Lq + Lk], BF16, tag="qkt2")
                qt2 = qkt2[:, 0:Lq]
                kt2 = qkt2[:, Lq:Lq + Lk]
                for s0, s1 in _chunks(Lq + Lk):
                    ps = pssmall.tile([P, 512], F32, tag="small")
                    # fused rhs: columns [0,Lq) from qts, [Lq,Lq+Lk) from kts
                    if s1 <= Lq:
                        rhs = qts[:, c, s0:s1]
                    elif s0 >= Lq:
                        rhs = kts[:, c, s0 - Lq:s1 - Lq]
                    else:
                        rhs = None
                    if rhs is not None:
                        nc.tensor.matmul(
                            out=ps[:, 0:s1 - s0], lhsT=w2s[:, c, :], rhs=rhs,
                            start=True, stop=True)
                        nc.scalar.copy(qkt2[:, s0:s1], ps[:, 0:s1 - s0])
                    else:
                        mid = Lq - s0
                        nc.tensor.matmul(
                            out=ps[:, 0:mid], lhsT=w2s[:, c, :],
                            rhs=qts[:, c, s0:Lq], start=True, stop=True)
                        nc.tensor.matmul(
                            out=ps[:, mid:s1 - s0], lhsT=w2s[:, c, :],
                            rhs=kts[:, c, 0:s1 - Lq], start=True, stop=True)
                        nc.scalar.copy(qkt2[:, s0:s1], ps[:, 0:s1 - s0])

                # v projection (keys compacted: only validity col needed)
                v2 = vaug.tile([P, ntk, 130], BF16)
                for t in range(ntk):
                    ps = pssmall.tile([P, P], F32, tag="small")
                    nc.tensor.matmul(
                        out=ps[:],
                        lhsT=vts[:, c, t * P:(t + 1) * P],
                        rhs=w2s[:, c, :],
                        start=True, stop=True,
                    )
                    base = v2[:, t, 0:64]
                    vt_out = bass.AP(
                        tensor=base.tensor, offset=base.offset,
                        ap=[list(base.ap[0]), [65, 2], [1, 64]])
                    nc.scalar.copy(
                        vt_out, ps[:].rearrange("p (two d) -> p two d", two=2))
                # denominator "ones" columns = slot-validity mask
                nc.vector.tensor_copy(v2[:, :, 64], kms[:, :])
                nc.vector.tensor_copy(v2[:, :, 129], kms[:, :])

                for hf in range(2):
                    h = 2 * c + hf
                    hq = qt2[64 * hf:64 * hf + 64, :]
                    hk = kt2[64 * hf:64 * hf + 64, :]
                    # scores (transposed, [k, q]) + exp -> P (bf16)
                    pt = ppool.tile([P, ntk, Lq], BF16)
                    for t in range(ntk):
                        sps = psbig.tile([P, Lq], F32, tag="big")
                        for s0, s1 in _chunks(Lq):
                            nc.tensor.matmul(
                                out=sps[:, s0:s1],
                                lhsT=hk[:, t * P:(t + 1) * P],
                                rhs=hq[:, s0:s1],
                                start=True, stop=True,
                            )
                        nc.scalar.activation(
                            out=pt[:, t, :], in_=sps[:],
                            func=mybir.ActivationFunctionType.Exp,
                            scale=0.125,
                        )
                    # PV: out[65, q] accumulated over k-tiles, wide rhs
                    pv = pspv.tile([65, Lq], F32)
                    for kt in range(ntk):
                        for s0, s1 in _chunks(Lq):
                            nc.tensor.matmul(
                                out=pv[:, s0:s1],
                                lhsT=v2[:, kt, 65 * hf:65 * hf + 65],
                                rhs=pt[:, kt, s0:s1],
                                start=(kt == 0), stop=(kt == ntk - 1),
                            )
                    # fast evacuation: denom row + unnormalized C^T rows
                    dtmp = dtpool.tile([65, Lq], F32)
                    nc.vector.tensor_copy(dtmp[64:65, :], pv[64:65, :])
                    bi = next(i for i, (a, b, _) in enumerate(NORM_BATCHES)
                              if a <= h < b)
                    hrel = h - NORM_BATCHES[bi][0]
                    nc.gpsimd.dma_start(
                        out=dstacks[bi][hrel * ntq:(hrel + 1) * ntq, :],
                        in_=dtmp[64:65, :])
                    nc.vector.tensor_copy(ct[64 * hf:64 * hf + 64, c, :], pv[0:64, :])

                for bi, (_, _, bc_) in enumerate(NORM_BATCHES):
                    if c == bc_ and bi < 2:
                        normalize_batch(bi)

            # tail: last normalize batch, then the split output projection
            normalize_batch(2)

            # part A: chunks 0-6 (normalized after batch 1) can overlap
            # the batch-2 normalize chain
            for t in range(ntq):
                ya0 = pssmall.tile([P, 512], F32, tag="small")
                ya1 = pssmall.tile([P, 512], F32, tag="small")
                final_mms(t, [ya0, ya1], list(range(7)))
                nc.vector.tensor_copy(ysum[:, t, 0:512], ya0[:])
                nc.vector.tensor_copy(ysum[:, t, 512:1024], ya1[:])
            # part B: chunk 7 + combine
            for t in range(ntq):
                yb0 = pssmall.tile([P, 512], F32, tag="small")
                yb1 = pssmall.tile([P, 512], F32, tag="small")
                final_mms(t, [yb0, yb1], [7])
                ys = ystage.tile([P, E], F32, tag="ys")
                nc.vector.tensor_add(ys[:, 0:512], yb0[:], ysum[:, t, 0:512])
                nc.vector.tensor_add(ys[:, 512:1024], yb1[:], ysum[:, t, 512:1024])
                nc.gpsimd.dma_start(out=Y[t * P:(t + 1) * P, :], in_=ys[:])

    nc.compile()
    return nc


def make_core_inputs(Q, K, V, HeadLinear, OutputLiner, QMask, KMask):
    """Host-side sharding/compaction.

    Returns (in_maps, qidxs, ntq, ntk).  qidxs[b] holds the query
    indices the DEVICE computes.  If the max valid-query count is only
    slightly above a 512 multiple (<= 64 over), the device is capped at
    that multiple (avoiding a whole extra 128-tile and remainder
    matmuls) and the few overflow queries are computed exactly on the
    host during gather (see _host_tail in kernel()).
    """
    bf16 = ml_dtypes.bfloat16
    qm = np.asarray(QMask).astype(bool)
    km = np.asarray(KMask).astype(bool)
    qidxs = [np.nonzero(qm[b])[0] for b in range(B)]
    kidxs = [np.nonzero(km[b])[0] for b in range(B)]
    maxq = max(len(ix) for ix in qidxs)
    qcap = maxq
    if maxq > 512 and maxq % 512 <= 64:
        qcap = (maxq // 512) * 512
    qidxs = [ix[:qcap] for ix in qidxs]
    ntq = max(1, math.ceil(max(len(ix) for ix in qidxs) / P))
    ntk = max(1, math.ceil(max(len(ix) for ix in kidxs) / P))
    Lq, Lk = ntq * P, ntk * P

    w2 = np.zeros((P, NCH, P), dtype=np.float32)
    hl = np.asarray(HeadLinear, dtype=np.float32)
    for c in range(NCH):
        w2[0:64, c, 0:64] = hl[2 * c]
        w2[64:128, c, 64:128] = hl[2 * c + 1]
    w2b = w2.astype(bf16)
    ob = np.asarray(OutputLiner, dtype=np.float32).astype(bf16)

    in_maps = []
    for b in range(B):
        qi, ki = qidxs[b], kidxs[b]
        qc = np.zeros((Lq, E), dtype=np.float32)
        qc[:len(qi)] = np.asarray(Q[b], dtype=np.float32)[qi]
        kc = np.zeros((Lk, E), dtype=np.float32)
        kc[:len(ki)] = np.asarray(K[b], dtype=np.float32)[ki]
        vc = np.zeros((Lk, E), dtype=np.float32)
        vc[:len(ki)] = np.asarray(V[b], dtype=np.float32)[ki]
        kmc = np.zeros(Lk, dtype=np.float32)
        kmc[:len(ki)] = 1.0
        in_maps.append({
            "QT": np.ascontiguousarray(qc.T.astype(bf16)),
            "KT": np.ascontiguousarray(kc.T.astype(bf16)),
            "VT": np.ascontiguousarray(vc.T.astype(bf16)),
            "W2": w2b, "OB": ob,
            "KM": np.ascontiguousarray(kmc.reshape(ntk, P).T),
        })
    return in_maps, qidxs, ntq, ntk


_NC_CACHE = {}


def _get_nc(ntq, ntk):
    if (ntq, ntk) not in _NC_CACHE:
        _NC_CACHE[(ntq, ntk)] = build_bass(ntq, ntk)
    return _NC_CACHE[(ntq, ntk)]


def _host_tail(Q, K, V, HeadLinear, OutputLiner, KMask, b, tidx):
    """Exact fp32 attention for a few overflow queries of batch b."""
    hl = np.asarray(HeadLinear, dtype=np.float32)
    ob = np.asarray(OutputLiner, dtype=np.float32)
    ki = np.nonzero(np.asarray(KMask[b]).astype(bool))[0]
    q = np.asarray(Q[b], dtype=np.float32)[tidx]
    kk = np.asarray(K[b], dtype=np.float32)[ki]
    vv = np.asarray(V[b], dtype=np.float32)[ki]
    outs = []
    for h in range(H):
        sl = slice(h * D, (h + 1) * D)
        qh = q[:, sl] @ hl[h]
        kh = kk[:, sl] @ hl[h]
        vh = vv[:, sl] @ hl[h]
        s = (qh @ kh.T) / np.float32(np.sqrt(D))
        s -= s.max(axis=1, keepdims=True)
        p = np.exp(s)
        p /= p.sum(axis=1, keepdims=True)
        outs.append(p @ vh)
    return np.concatenate(outs, axis=1) @ ob


def kernel(Q, K, V, HeadLinear, OutputLiner, QMask, KMask):
    from concourse.bass_utils import run_bass_kernel_spmd

    in_maps, qidxs, ntq, ntk = make_core_inputs(
        Q, K, V, HeadLinear, OutputLiner, QMask, KMask)
    nc = _get_nc(ntq, ntk)
    res = run_bass_kernel_spmd(nc, in_maps, list(range(B)))
    out = np.zeros((B, L, E), dtype=np.float32)
    qm = np.asarray(QMask).astype(bool)
    for b in range(B):
        yc = np.asarray(res.results[b]["Y"])
        out[b][qidxs[b]] = yc[:len(qidxs[b])]
        full = np.nonzero(qm[b])[0]
        tidx = full[len(qidxs[b]):]
        if len(tidx):
            out[b][tidx] = _host_tail(
                Q, K, V, HeadLinear, OutputLiner, KMask, b, tidx)
    return out



# revision 68
# speedup vs baseline: 1.2804x; 1.2804x over previous
"""Trainium2 Bass kernel for the nn_MultiHeadAttention problem.

Data-parallel over batch: each of the 8 NeuronCores processes one batch
element independently (no collectives).

Mask compaction: the host gathers only the valid query/key positions
(QMask/KMask true), padded to a multiple of 128, and scatters the
output back (masked query rows are exactly zero in the reference).
With ~50% random masks this cuts the attention work ~4x.  If the max
query count only slightly exceeds a 512 multiple, the device is capped
there and the few overflow queries are computed exactly on the host.

The host also applies the per-head HeadLinear projection to Q/K/V (it
is O(L*E*D), tiny next to the O(L^2*E) attention), so the device does
only: scores, exp, PV, softmax-normalize, and the output projection.

Per-core dataflow (E=1024, H=16, D=64; Lq=ntq*128 queries, Lk=ntk*128
keys after compaction; e-chunks of 128 = 2 heads):

  scores: for each k-tile, the two heads' score matmuls (K=64) are
        issued back-to-back into one [128,2,Lq] 2-bank psum tile; the
        auto-derived tile_positions (0,0)/(64,0) make them execute
        CONCURRENTLY in the two row-halves of the PE array (64x128
        row tiling).  One fused exp ACT (N=2*Lq) per k-tile covers
        both heads -> P tiles (bf16).  No max subtraction (|s|/8 <~
        13); pad keys have zero V rows and validity 0.
  PV:   out[65,q] psum = sum_t vslab_slice.T @ P_slice; vslab holds
        the host-projected V with a per-key validity column per head,
        so row 64 is the masked softmax denominator.
  norm: denom rows -> dstack via SBUF staging + DMA shuffle;
        reciprocals in two batches; DRAM-bounce broadcast + one DVE
        multiply per head normalizes ct in place.
  final: the output projection is split: part 1 (k-chunks 0-3, valid
        once norm batch 0 lands) is interleaved into the scalar-bound
        main-loop windows of chunks 4-7 to keep the PE busy (HAM stays
        at 8/8); part 2 (k-chunks 4-7) runs as the tail.
"""

import math
import os
import sys

import numpy as np

try:
    import concourse  # noqa: F401
except ImportError:  # pragma: no cover
    for _p in ("/opt/trn_rl_repo", os.path.expanduser("~/.axon_site/_ro/trn_rl_repo")):
        if os.path.isdir(_p) and _p not in sys.path:
            sys.path.insert(0, _p)

import ml_dtypes

import concourse.bass as bass
import concourse.tile as tile
from concourse import bacc, mybir

B, L, E, H, D = 8, 1024, 1024, 16, 64
P = 128          # partitions
NCH = E // P     # 8 e-chunks (2 heads each)
DV = D + 1       # per-head V columns + validity column
F32 = mybir.dt.float32
BF16 = mybir.dt.bfloat16

# final projection split: k-chunks 0-3 inside the main loop, 4-7 in the tail
FIN_SPLIT = 3


def build_bass(ntq, ntk):
    Lq, Lk = ntq * P, ntk * P
    nc = bacc.Bacc(None, target_bir_lowering=False, debug=False)

    QT = nc.declare_dram_parameter("QT", [E, Lq], BF16, isOutput=False)
    KT = nc.declare_dram_parameter("KT", [E, Lk], BF16, isOutput=False)
    VS = nc.declare_dram_parameter("VS", [Lk, H * DV], BF16, isOutput=False)
    OB = nc.declare_dram_parameter("OB", [E, E], BF16, isOutput=False)
    Y = nc.declare_dram_parameter("Y", [Lq, E], BF16, isOutput=True)
    rbounce = nc.dram_tensor("rbounce", [H, Lq], BF16)

    with tile.TileContext(nc) as tc:
        with (
            tc.tile_pool(name="singles", bufs=1) as singles,
            tc.tile_pool(name="ptpool", bufs=2) as ptpool,
            tc.tile_pool(name="ystage", bufs=2) as ystage,
            tc.tile_pool(name="bcpool", bufs=3) as bcpool,
            tc.tile_pool(name="dtpool", bufs=2) as dtpool,
            tc.tile_pool(name="scpool", bufs=2, space="PSUM") as scpool,
            tc.tile_pool(name="pvpool", bufs=1, space="PSUM") as pvpool,
            tc.tile_pool(name="pssmall", bufs=2, space="PSUM") as pssmall,
        ):
            # --- persistent SBUF tensors -------------------------------
            qts = singles.tile([P, NCH, Lq], BF16)
            kts = singles.tile([P, NCH, Lk], BF16)
            vst = singles.tile([P, ntk, H * DV], BF16)
            obs = singles.tile([P, NCH, E], BF16)
            ct = singles.tile([P, NCH, Lq], BF16)
            ysum = singles.tile([P, ntq, E], F32)
            dstacks = []
            rstacks = []
            rfs = []
            for c in range(NCH):
                ds = singles.tile([2 * ntq, P], F32, tag=f"ds{c}")
                rf = singles.tile([2 * ntq, P], F32, tag=f"rf{c}")
                rs = singles.tile([2 * ntq, P], BF16, tag=f"rs{c}")
                dstacks.append(ds)
                rfs.append(rf)
                rstacks.append(rs)

            # --- input DMAs + ACT table preload ------------------------
            # (the first real score matmuls run cold and open the HAM
            # clock gate themselves; a dummy tiny exp preloads the ACT
            # spline tables off the critical path)
            warm = singles.tile([P, 512], BF16)
            nc.vector.memset(warm[:], 0.0)
            nc.scalar.activation(
                out=warm[0:1, 0:8], in_=warm[0:1, 8:16],
                func=mybir.ActivationFunctionType.Exp, scale=0.125)
            ones = singles.tile([1, P], BF16)
            nc.vector.memset(ones[:], 1.0)
            rrow = singles.tile([1, 2, Lq], F32)
            rrowb = singles.tile([1, 2, Lq], BF16)
            for wi in range(6):
                wps = pssmall.tile([P, 512], F32, tag="small")
                nc.tensor.matmul(out=wps[:], lhsT=warm[:, 0:128], rhs=warm[:],
                                 start=True, stop=True)
            nc.sync.dma_start(out=qts[:, 0, :], in_=QT[0:P, :])
            nc.gpsimd.dma_start(out=kts[:, 0, :], in_=KT[0:P, :])
            for t in range(ntk):
                eng = nc.sync if t % 2 == 0 else nc.gpsimd
                eng.dma_start(out=vst[:, t, :], in_=VS[t * P:(t + 1) * P, :])
            for c in range(1, NCH):
                enq = nc.sync if c % 2 == 0 else nc.gpsimd
                enk = nc.gpsimd if c % 2 == 0 else nc.sync
                enq.dma_start(out=qts[:, c, :], in_=QT[c * P:(c + 1) * P, :])
                enk.dma_start(out=kts[:, c, :], in_=KT[c * P:(c + 1) * P, :])

            def load_obs(cs):
                for c in cs:
                    eng = nc.sync if c % 2 == 0 else nc.gpsimd
                    eng.dma_start(out=obs[:, c, :], in_=OB[c * P:(c + 1) * P, :])

            def normalize_chunk(c):
                # per-chunk: fast-approx recip of both heads'
                # denominators (bf16 output -> 18 bits is plenty),
                # bounce through DRAM, one broadcast read that lands
                # each head's recip row on its own 64 partitions, two
                # muls -- ct[:, c, :] is normalized ~3us after PV_c
                nc.vector.reciprocal_approx_fast(
                    out=rfs[c][:], in_=dstacks[c][:])
                with nc.allow_low_precision(reason="softmax recip bf16"):
                    nc.vector.tensor_copy(rstacks[c][:], rfs[c][:])
                eng = nc.sync if c % 2 == 0 else nc.gpsimd
                eng2 = nc.gpsimd if c % 2 == 0 else nc.sync
                eng.dma_start(out=rbounce[2 * c:2 * c + 2, :], in_=rstacks[c][:])
                bcs = bcpool.tile([P, Lq], BF16)
                for hf in range(2):
                    src = rbounce[2 * c + hf:2 * c + hf + 1, :]
                    bc_in = bass.AP(
                        tensor=src.tensor, offset=src.offset,
                        ap=[[0, 64], list(src.ap[-1])])
                    (eng if hf == 0 else eng2).dma_start(
                        out=bcs[64 * hf:64 * hf + 64, :], in_=bc_in)
                for hf in range(2):
                    sl = ct[64 * hf:64 * hf + 64, c, :]
                    nc.vector.tensor_mul(
                        sl, sl, bcs[64 * hf:64 * hf + 64, :])

            def final_mms(t, ytiles, crange, first=None, last=None):
                first = crange[0] if first is None else first
                last = crange[-1] if last is None else last
                for c in crange:
                    for eh in range(2):
                        nc.tensor.matmul(
                            out=ytiles[eh][:],
                            lhsT=ct[:, c, t * P:(t + 1) * P],
                            rhs=obs[:, c, 512 * eh:512 * (eh + 1)],
                            start=(c == first), stop=(c == last),
                        )

            # --- main loop over e-chunks (2 heads each) ----------------
            # Software-pipelined: scores+exp for chunk c are emitted
            # ahead of the PV/evac of chunk c-1, so the scalar engine
            # (the pacing engine) never waits on a chunk boundary.
            pts = [None, None]

            def emit_scores(c, t):
                # the two heads' K=64 matmuls back-to-back -> concurrent
                # row-tiled execution, then one fused exp on both banks
                sc = scpool.tile([P, 2, Lq], F32, tag="sc")
                nc.tensor.matmul(
                    out=sc[:, 0, :],
                    lhsT=kts[0:64, c, t * P:(t + 1) * P],
                    rhs=qts[0:64, c, :], start=True, stop=True)
                nc.tensor.matmul(
                    out=sc[:, 1, :],
                    lhsT=kts[64:128, c, t * P:(t + 1) * P],
                    rhs=qts[64:128, c, :], start=True, stop=True)
                nc.scalar.activation(
                    out=pts[c % 2][:, :, t, :], in_=sc[:, :, :],
                    func=mybir.ActivationFunctionType.Exp,
                    scale=0.125,
                )

            def emit_pv(c, t, pvt):
                for hf in range(2):
                    h = 2 * c + hf
                    nc.tensor.matmul(
                        out=pvt[0:DV, hf, :],
                        lhsT=vst[:, t, h * DV:(h + 1) * DV],
                        rhs=pts[c % 2][:, hf, t, :],
                        start=(t == 0), stop=(t == ntk - 1),
                    )

            def emit_evac(c, pvt):
                # denom rows first (they gate the normalize chain),
                # then the unnormalized C^T rows
                dtmp = dtpool.tile([DV, 2, Lq], F32)
                nc.vector.tensor_copy(dtmp[64:65, :, :], pvt[64:65, :, :])
                eng = nc.gpsimd if c % 2 == 0 else nc.sync
                eng.dma_start(out=dstacks[c][:], in_=dtmp[64:65, :, :])
                for hf in range(2):
                    nc.vector.tensor_copy(
                        ct[64 * hf:64 * hf + 64, c, :], pvt[0:64, hf, :])
                normalize_chunk(c)

            pvt_prev = None
            for c in range(NCH):
                pt_cur = ptpool.tile([P, 2, ntk, Lq], BF16, tag="pt")
                pts[c % 2] = pt_cur
                pvt = pvpool.tile([P, 2, Lq], F32, tag="pv")
                # interleaving order: every PV of c-1 is dependency-free
                # at this point (its exps ran last window), while scores
                # tile t of chunk c must wait for exp t-2 to free its
                # psum buffer.  Slot ready work between the waiting
                # score matmuls so the next chunk's first scores issue
                # before the scalar engine finishes this chunk's exps.
                # interleave: 2 score tiles of c, then PV of c-1 in
                # chunks so the PE never runs dry while the scalar
                # engine chews the fused exps
                emit_scores(c, 0)
                emit_scores(c, 1)
                if c > 0:
                    for t in range(ntk):
                        emit_pv(c - 1, t, pvt_prev)
                for t in range(2, ntk):
                    emit_scores(c, t)
                if c > 0:
                    emit_evac(c - 1, pvt_prev)
                if c == 1:
                    load_obs(range(0, 4))
                if c == 2:
                    load_obs(range(4, NCH))
                # final part 1 (k-chunks 0-3) interleaved into the
                # scalar-bound windows of chunks 4-7
                if c >= NCH - ntq:
                    t = c - (NCH - ntq)
                    ya0 = pssmall.tile([P, 512], F32, tag="small")
                    ya1 = pssmall.tile([P, 512], F32, tag="small")
                    final_mms(t, [ya0, ya1], list(range(FIN_SPLIT)))
                    nc.vector.tensor_copy(ysum[:, t, 0:512], ya0[:])
                    nc.vector.tensor_copy(ysum[:, t, 512:1024], ya1[:])
                pvt_prev = pvt

            # tail: the last chunk's PV runs in psum borrowed from the
            # (now idle) scores pool; its normalize skips the DRAM
            # bounce entirely -- reciprocal straight from psum, then
            # the recip row is broadcast across partitions with two
            # K=1 matmuls on the idle PE (~3us instead of ~9us of DMA
            # hops).  Final part-2 c4-c6 accumulation overlaps it.
            cl = NCH - 1
            pvt_last = pvt_prev
            for t in range(ntk):
                emit_pv(cl, t, pvt_last)
            dlast = dtpool.tile([DV, 2, Lq], F32)
            dlast0 = singles.tile([1, 2, Lq], F32)
            nc.vector.tensor_copy(dlast[64:65, :, :], pvt_last[64:65, :, :])
            nc.gpsimd.dma_start(out=dlast0[:], in_=dlast[64:65, :, :])
            nc.vector.reciprocal_approx_fast(out=rrow[:], in_=dlast0[:])
            with nc.allow_low_precision(reason="softmax recip bf16"):
                nc.vector.tensor_copy(rrowb[:], rrow[:])
            # ct evacuation of the last chunk on the (now idle) scalar
            # engine, keeping the DVE free for the recip chain
            for hf in range(2):
                nc.scalar.copy(
                    ct[64 * hf:64 * hf + 64, cl, :], pvt_last[0:64, hf, :])

            # final part 2: all four q-tiles accumulate concurrently
            # (t0/t1 in the freed scores pool, t2 in the pv pool, t3 in
            # pssmall), c4-c6 matmuls first so they overlap the last
            # chunk's normalize chain; the broadcast-normalize runs
            # col-tiled into a single pssmall bank.
            ybs = {}
            for t in range(min(2, ntq)):
                yb = scpool.tile([P, 2, Lq], F32, tag="sc")
                ybs[t] = yb
                final_mms(t, [yb[:, 0, :], yb[:, 1, :]],
                          list(range(FIN_SPLIT, NCH - 1)),
                          first=FIN_SPLIT, last=NCH - 1)
            bcp = pssmall.tile([P, Lq], F32, tag="small")
            for hf in range(2):
                nc.tensor.matmul(
                    out=bcp[64 * hf:64 * hf + 64, :],
                    lhsT=ones[0:1, 64 * hf:64 * hf + 64],
                    rhs=rrowb[0:1, hf, :], start=True, stop=True)
            if ntq > 2:
                yb = pvpool.tile([P, 2, Lq], F32, tag="pv")
                ybs[2] = yb
                final_mms(2, [yb[:, 0, :], yb[:, 1, :]],
                          list(range(FIN_SPLIT, NCH - 1)),
                          first=FIN_SPLIT, last=NCH - 1)
            for hf in range(2):
                sl = ct[64 * hf:64 * hf + 64, cl, :]
                nc.vector.tensor_mul(sl, sl, bcp[64 * hf:64 * hf + 64, :])
            if ntq > 3:
                yb0 = pssmall.tile([P, 512], F32, tag="small")
                yb1 = pssmall.tile([P, 512], F32, tag="small")
                ybs[3] = None
                final_mms(3, [yb0, yb1], list(range(FIN_SPLIT, NCH - 1)),
                          first=FIN_SPLIT, last=NCH - 1)
                yb3 = [yb0, yb1]
            for t in range(ntq):
                yb = ybs[t]
                ytiles = yb3 if yb is None else [yb[:, 0, :], yb[:, 1, :]]
                final_mms(t, ytiles, [NCH - 1],
                          first=FIN_SPLIT, last=NCH - 1)
                ys = ystage.tile([P, E], BF16, tag="ys")
                with nc.allow_low_precision(reason="bf16 output"):
                    nc.vector.tensor_add(
                        ys[:, 0:512], ytiles[0][:], ysum[:, t, 0:512])
                    nc.vector.tensor_add(
                        ys[:, 512:1024], ytiles[1][:], ysum[:, t, 512:1024])
                eng = nc.gpsimd if t % 2 == 0 else nc.sync
                eng.dma_start(out=Y[t * P:(t + 1) * P, :], in_=ys[:])

    nc.compile()
    return nc


def make_core_inputs(Q, K, V, HeadLinear, OutputLiner, QMask, KMask):
    """Host-side sharding/compaction/projection.

    Returns (in_maps, qidxs, ntq, ntk).  qidxs[b] holds the query
    indices the DEVICE computes.  If the max valid-query count is only
    slightly above a 512 multiple (<= 64 over), the device is capped at
    that multiple and the few overflow queries are computed exactly on
    the host during gather (see _host_tail in kernel()).

    The HeadLinear projection is applied on the host (fp32), and V is
    packed as [Lk, H, 65] with a per-key validity column per head.
    """
    bf16 = ml_dtypes.bfloat16
    qm = np.asarray(QMask).astype(bool)
    km = np.asarray(KMask).astype(bool)
    qidxs = [np.nonzero(qm[b])[0] for b in range(B)]
    kidxs = [np.nonzero(km[b])[0] for b in range(B)]
    maxq = max(len(ix) for ix in qidxs)
    qcap = maxq
    if maxq > 512 and maxq % 512 <= 64:
        qcap = (maxq // 512) * 512
    qidxs = [ix[:qcap] for ix in qidxs]
    ntq = max(1, math.ceil(max(len(ix) for ix in qidxs) / P))
    ntk = max(1, math.ceil(max(len(ix) for ix in kidxs) / P))
    Lq, Lk = ntq * P, ntk * P

    hl = np.asarray(HeadLinear, dtype=np.float32)
    ob = np.asarray(OutputLiner, dtype=np.float32).astype(bf16)

    def proj(x):
        # x [n, E] -> [H*D, n] head-major (chunk c rows = heads 2c,2c+1)
        xh = x.reshape(-1, H, D).transpose(1, 0, 2)  # [H, n, D]
        ph = np.matmul(xh, hl)                       # [H, n, D]
        return ph.transpose(0, 2, 1).reshape(E, -1)

    in_maps = []
    for b in range(B):
        qi, ki = qidxs[b], kidxs[b]
        qc = np.zeros((Lq, E), dtype=np.float32)
        qc[:len(qi)] = np.asarray(Q[b], dtype=np.float32)[qi]
        kc = np.zeros((Lk, E), dtype=np.float32)
        kc[:len(ki)] = np.asarray(K[b], dtype=np.float32)[ki]
        vc = np.zeros((Lk, E), dtype=np.float32)
        vc[:len(ki)] = np.asarray(V[b], dtype=np.float32)[ki]

        vh = np.matmul(vc.reshape(Lk, H, D).transpose(1, 0, 2), hl)  # [H,Lk,D]
        vs = np.zeros((Lk, H, DV), dtype=np.float32)
        vs[:, :, 0:D] = vh.transpose(1, 0, 2)
        vs[:len(ki), :, D] = 1.0

        in_maps.append({
            "QT": np.ascontiguousarray(proj(qc).astype(bf16)),
            "KT": np.ascontiguousarray(proj(kc).astype(bf16)),
            "VS": np.ascontiguousarray(vs.reshape(Lk, H * DV).astype(bf16)),
            "OB": ob,
        })
    return in_maps, qidxs, ntq, ntk


_NC_CACHE = {}


def _get_nc(ntq, ntk):
    if (ntq, ntk) not in _NC_CACHE:
        _NC_CACHE[(ntq, ntk)] = build_bass(ntq, ntk)
    return _NC_CACHE[(ntq, ntk)]


def _host_tail(Q, K, V, HeadLinear, OutputLiner, KMask, b, tidx):
    """Exact fp32 attention for a few overflow queries of batch b."""
    hl = np.asarray(HeadLinear, dtype=np.float32)
    ob = np.asarray(OutputLiner, dtype=np.float32)
    ki = np.nonzero(np.asarray(KMask[b]).astype(bool))[0]
    q = np.asarray(Q[b], dtype=np.float32)[tidx]
    kk = np.asarray(K[b], dtype=np.float32)[ki]
    vv = np.asarray(V[b], dtype=np.float32)[ki]
    outs = []
    for h in range(H):
        sl = slice(h * D, (h + 1) * D)
        qh = q[:, sl] @ hl[h]
        kh = kk[:, sl] @ hl[h]
        vh = vv[:, sl] @ hl[h]
        s = (qh @ kh.T) / np.float32(np.sqrt(D))
        s -= s.max(axis=1, keepdims=True)
        p = np.exp(s)
        p /= p.sum(axis=1, keepdims=True)
        outs.append(p @ vh)
    return np.concatenate(outs, axis=1) @ ob


def kernel(Q, K, V, HeadLinear, OutputLiner, QMask, KMask):
    from concourse.bass_utils import run_bass_kernel_spmd

    in_maps, qidxs, ntq, ntk = make_core_inputs(
        Q, K, V, HeadLinear, OutputLiner, QMask, KMask)
    nc = _get_nc(ntq, ntk)
    res = run_bass_kernel_spmd(nc, in_maps, list(range(B)))
    out = np.zeros((B, L, E), dtype=np.float32)
    qm = np.asarray(QMask).astype(bool)
    for b in range(B):
        yc = np.asarray(res.results[b]["Y"]).astype(np.float32)
        out[b][qidxs[b]] = yc[:len(qidxs[b])]
        full = np.nonzero(qm[b])[0]
        tidx = full[len(qidxs[b]):]
        if len(tidx):
            out[b][tidx] = _host_tail(
                Q, K, V, HeadLinear, OutputLiner, KMask, b, tidx)
    return out


# revision 85
# speedup vs baseline: 1.4834x; 1.1586x over previous
"""Trainium2 Bass kernel for the nn_MultiHeadAttention problem.

Data-parallel over batch: each of the 8 NeuronCores processes one batch
element independently (no collectives).

Mask compaction: the host gathers only the valid query/key positions
(QMask/KMask true), padded to a multiple of 128, and scatters the
output back (masked query rows are exactly zero in the reference).
With ~50% random masks this cuts the attention work ~4x.  If the max
query count only slightly exceeds a 512 multiple, the device is capped
there and the few overflow queries are computed exactly on the host.

The host also applies the per-head HeadLinear projection to Q/K/V (it
is O(L*E*D), tiny next to the O(L^2*E) attention), so the device does
only: scores, exp, PV, softmax-normalize, and the output projection.

Per-core dataflow (E=1024, H=16, D=64; Lq=ntq*128 queries, Lk=ntk*128
keys after compaction; e-chunks of 128 = 2 heads):

  scores: for each k-tile, the two heads' score matmuls (K=64) are
        issued back-to-back into one [128,2,Lq] 2-bank psum tile; the
        auto-derived tile_positions (0,0)/(64,0) make them execute
        CONCURRENTLY in the two row-halves of the PE array (64x128
        row tiling).  One fused exp ACT (N=2*Lq) per k-tile covers
        both heads -> P tiles (bf16).  No max subtraction (|s|/8 <~
        13); pad keys have zero V rows and validity 0.
  PV:   out[65,q] psum = sum_t vslab_slice.T @ P_slice; vslab holds
        the host-projected V with a per-key validity column per head,
        so row 64 is the masked softmax denominator.
  norm: denom rows -> dstack via SBUF staging + DMA shuffle;
        reciprocals in two batches; DRAM-bounce broadcast + one DVE
        multiply per head normalizes ct in place.
  final: the output projection is split: part 1 (k-chunks 0-3, valid
        once norm batch 0 lands) is interleaved into the scalar-bound
        main-loop windows of chunks 4-7 to keep the PE busy (HAM stays
        at 8/8); part 2 (k-chunks 4-7) runs as the tail.
"""

import math
import os
import sys

import numpy as np

try:
    import concourse  # noqa: F401
except ImportError:  # pragma: no cover
    for _p in ("/opt/trn_rl_repo", os.path.expanduser("~/.axon_site/_ro/trn_rl_repo")):
        if os.path.isdir(_p) and _p not in sys.path:
            sys.path.insert(0, _p)

import ml_dtypes

import concourse.bass as bass
import concourse.tile as tile
from concourse import bacc, mybir

B, L, E, H, D = 8, 1024, 1024, 16, 64
P = 128          # partitions
NCH = E // P     # 8 e-chunks (2 heads each)
DV = D + 1       # per-head V columns + validity column
F32 = mybir.dt.float32
BF16 = mybir.dt.bfloat16

# final projection split: k-chunks 0-3 inside the main loop, 4-7 in the tail
FIN_SPLIT = 3


def build_bass(ntq, ntk):
    Lq, Lk = ntq * P, ntk * P
    nc = bacc.Bacc(None, target_bir_lowering=False, debug=False)

    QT = nc.declare_dram_parameter("QT", [E, Lq], BF16, isOutput=False)
    KT = nc.declare_dram_parameter("KT", [E, Lk], BF16, isOutput=False)
    VS = nc.declare_dram_parameter("VS", [Lk, H * DV], BF16, isOutput=False)
    OB = nc.declare_dram_parameter("OB", [E, E], BF16, isOutput=False)
    EYE = nc.declare_dram_parameter("EYE", [P, P], BF16, isOutput=False)
    Y = nc.declare_dram_parameter("Y", [Lq, E], BF16, isOutput=True)
    rbounce = nc.dram_tensor("rbounce", [H, Lq], BF16)

    with tile.TileContext(nc) as tc:
        with (
            tc.tile_pool(name="singles", bufs=1) as singles,
            tc.tile_pool(name="ptpool", bufs=2) as ptpool,
            tc.tile_pool(name="ystage", bufs=4) as ystage,
            tc.tile_pool(name="bcpool", bufs=3) as bcpool,
            tc.tile_pool(name="dtpool", bufs=2) as dtpool,
            tc.tile_pool(name="scpool", bufs=2, space="PSUM") as scpool,
            tc.tile_pool(name="pvpool", bufs=1, space="PSUM") as pvpool,
            tc.tile_pool(name="pssmall", bufs=2, space="PSUM") as pssmall,
        ):
            # --- persistent SBUF tensors -------------------------------
            qts = singles.tile([P, NCH, Lq], BF16)
            kts = singles.tile([P, NCH, Lk], BF16)
            vst = singles.tile([P, ntk, H * DV], BF16)
            obs = singles.tile([P, NCH, E], BF16)
            ct = singles.tile([P, NCH, Lq], BF16)
            ysum = singles.tile([P, ntq, E], BF16)
            eyes = singles.tile([P, P], BF16)
            dstacks = []
            rstacks = []
            rfs = []
            for c in range(NCH):
                ds = singles.tile([2 * ntq, P], F32, tag=f"ds{c}")
                rf = singles.tile([2 * ntq, P], F32, tag=f"rf{c}")
                rs = singles.tile([2 * ntq, P], BF16, tag=f"rs{c}")
                dstacks.append(ds)
                rfs.append(rf)
                rstacks.append(rs)

            # --- input DMAs + ACT table preload ------------------------
            # (the first real score matmuls run cold and open the HAM
            # clock gate themselves; a dummy tiny exp preloads the ACT
            # spline tables off the critical path)
            warm = singles.tile([P, 512], BF16)
            nc.vector.memset(warm[:], 0.0)
            nc.scalar.activation(
                out=warm[0:1, 0:8], in_=warm[0:1, 8:16],
                func=mybir.ActivationFunctionType.Exp, scale=0.125)
            ones = singles.tile([1, P], BF16)
            nc.vector.memset(ones[:], 1.0)
            rrow = singles.tile([1, 2, Lq], F32)
            rrowb = singles.tile([1, 2, Lq], BF16)
            for wi in range(6):
                wps = pssmall.tile([P, 512], F32, tag="small")
                nc.tensor.matmul(out=wps[:], lhsT=warm[:, 0:128], rhs=warm[:],
                                 start=True, stop=True)
            nc.sync.dma_start(out=qts[:, 0, :], in_=QT[0:P, :])
            nc.gpsimd.dma_start(out=kts[:, 0, :], in_=KT[0:P, :])
            for t in range(ntk):
                eng = nc.sync if t % 2 == 0 else nc.gpsimd
                eng.dma_start(out=vst[:, t, :], in_=VS[t * P:(t + 1) * P, :])
            for c in range(1, NCH):
                enq = nc.sync if c % 2 == 0 else nc.gpsimd
                enk = nc.gpsimd if c % 2 == 0 else nc.sync
                enq.dma_start(out=qts[:, c, :], in_=QT[c * P:(c + 1) * P, :])
                enk.dma_start(out=kts[:, c, :], in_=KT[c * P:(c + 1) * P, :])

            nc.gpsimd.dma_start(out=eyes[:], in_=EYE[:])

            def load_obs(cs):
                for c in cs:
                    eng = nc.sync if c % 2 == 0 else nc.gpsimd
                    eng.dma_start(out=obs[:, c, :], in_=OB[c * P:(c + 1) * P, :])

            def normalize_chunk(c):
                # per-chunk: fast-approx recip of both heads'
                # denominators (bf16 output -> 18 bits is plenty),
                # bounce through DRAM, one broadcast read that lands
                # each head's recip row on its own 64 partitions, two
                # muls -- ct[:, c, :] is normalized ~3us after PV_c
                nc.vector.reciprocal_approx_fast(
                    out=rfs[c][:], in_=dstacks[c][:])
                with nc.allow_low_precision(reason="softmax recip bf16"):
                    nc.vector.tensor_copy(rstacks[c][:], rfs[c][:])
                eng = nc.sync if c % 2 == 0 else nc.gpsimd
                eng2 = nc.gpsimd if c % 2 == 0 else nc.sync
                eng.dma_start(out=rbounce[2 * c:2 * c + 2, :], in_=rstacks[c][:])
                bcs = bcpool.tile([P, Lq], BF16)
                for hf in range(2):
                    src = rbounce[2 * c + hf:2 * c + hf + 1, :]
                    bc_in = bass.AP(
                        tensor=src.tensor, offset=src.offset,
                        ap=[[0, 64], list(src.ap[-1])])
                    (eng if hf == 0 else eng2).dma_start(
                        out=bcs[64 * hf:64 * hf + 64, :], in_=bc_in)
                for hf in range(2):
                    sl = ct[64 * hf:64 * hf + 64, c, :]
                    nc.vector.tensor_mul(
                        sl, sl, bcs[64 * hf:64 * hf + 64, :])

            def final_mms(t, ytiles, crange, first=None, last=None):
                first = crange[0] if first is None else first
                last = crange[-1] if last is None else last
                for c in crange:
                    for eh in range(2):
                        nc.tensor.matmul(
                            out=ytiles[eh][:],
                            lhsT=ct[:, c, t * P:(t + 1) * P],
                            rhs=obs[:, c, 512 * eh:512 * (eh + 1)],
                            start=(c == first), stop=(c == last),
                        )

            # --- main loop over e-chunks (2 heads each) ----------------
            # Software-pipelined: scores+exp for chunk c are emitted
            # ahead of the PV/evac of chunk c-1, so the scalar engine
            # (the pacing engine) never waits on a chunk boundary.
            pts = [None, None]

            def emit_scores(c, t):
                # the two heads' K=64 matmuls back-to-back -> concurrent
                # row-tiled execution, then one fused exp on both banks
                sc = scpool.tile([P, 2, Lq], F32, tag="sc")
                nc.tensor.matmul(
                    out=sc[:, 0, :],
                    lhsT=kts[0:64, c, t * P:(t + 1) * P],
                    rhs=qts[0:64, c, :], start=True, stop=True)
                nc.tensor.matmul(
                    out=sc[:, 1, :],
                    lhsT=kts[64:128, c, t * P:(t + 1) * P],
                    rhs=qts[64:128, c, :], start=True, stop=True)
                nc.scalar.activation(
                    out=pts[c % 2][:, :, t, :], in_=sc[:, :, :],
                    func=mybir.ActivationFunctionType.Exp,
                    scale=0.125,
                )

            def emit_pv(c, t, pvt):
                for hf in range(2):
                    h = 2 * c + hf
                    nc.tensor.matmul(
                        out=pvt[0:DV, hf, :],
                        lhsT=vst[:, t, h * DV:(h + 1) * DV],
                        rhs=pts[c % 2][:, hf, t, :],
                        start=(t == 0), stop=(t == ntk - 1),
                    )

            def emit_evac(c, pvt):
                # denom rows first (they gate the normalize chain),
                # then the unnormalized C^T rows
                dtmp = dtpool.tile([DV, 2, Lq], F32)
                nc.vector.tensor_copy(dtmp[64:65, :, :], pvt[64:65, :, :])
                eng = nc.gpsimd if c % 2 == 0 else nc.sync
                eng.dma_start(out=dstacks[c][:], in_=dtmp[64:65, :, :])
                for hf in range(2):
                    nc.vector.tensor_copy(
                        ct[64 * hf:64 * hf + 64, c, :], pvt[0:64, hf, :])
                normalize_chunk(c)

            pvt_prev = None
            for c in range(NCH):
                pt_cur = ptpool.tile([P, 2, ntk, Lq], BF16, tag="pt")
                pts[c % 2] = pt_cur
                pvt = pvpool.tile([P, 2, Lq], F32, tag="pv")
                # interleaving order: every PV of c-1 is dependency-free
                # at this point (its exps ran last window), while scores
                # tile t of chunk c must wait for exp t-2 to free its
                # psum buffer.  Slot ready work between the waiting
                # score matmuls so the next chunk's first scores issue
                # before the scalar engine finishes this chunk's exps.
                # interleave: 2 score tiles of c, then PV of c-1 in
                # chunks so the PE never runs dry while the scalar
                # engine chews the fused exps
                emit_scores(c, 0)
                emit_scores(c, 1)
                if c > 0:
                    for t in range(ntk):
                        emit_pv(c - 1, t, pvt_prev)
                for t in range(2, ntk):
                    emit_scores(c, t)
                if c > 0:
                    emit_evac(c - 1, pvt_prev)
                if c == 3:
                    load_obs(range(0, 4))
                if c == 5:
                    load_obs(range(4, NCH))
                # final part 1 (k-chunks 0-3) interleaved into the
                # scalar-bound windows of chunks 4-7
                if c >= NCH - ntq:
                    t = c - (NCH - ntq)
                    ya0 = pssmall.tile([P, 512], F32, tag="small")
                    ya1 = pssmall.tile([P, 512], F32, tag="small")
                    final_mms(t, [ya0, ya1], list(range(FIN_SPLIT)))
                    with nc.allow_low_precision(reason="bf16 ysum"):
                        nc.vector.tensor_copy(ysum[:, t, 0:512], ya0[:])
                        nc.vector.tensor_copy(ysum[:, t, 512:1024], ya1[:])
                pvt_prev = pvt

            # tail: the last chunk's PV runs in psum borrowed from the
            # (now idle) scores pool; its normalize skips the DRAM
            # bounce entirely -- reciprocal straight from psum, then
            # the recip row is broadcast across partitions with two
            # K=1 matmuls on the idle PE (~3us instead of ~9us of DMA
            # hops).  Final part-2 c4-c6 accumulation overlaps it.
            cl = NCH - 1
            pvt_last = pvt_prev
            for t in range(ntk):
                emit_pv(cl, t, pvt_last)
            dlast = dtpool.tile([DV, 2, Lq], F32)
            dlast0 = singles.tile([1, 2, Lq], F32)
            nc.vector.tensor_copy(dlast[64:65, :, :], pvt_last[64:65, :, :])
            nc.gpsimd.dma_start(out=dlast0[:], in_=dlast[64:65, :, :])
            nc.vector.reciprocal_approx_fast(out=rrow[:], in_=dlast0[:])
            with nc.allow_low_precision(reason="softmax recip bf16"):
                nc.vector.tensor_copy(rrowb[:], rrow[:])
            # ct evacuation of the last chunk on the (now idle) scalar
            # engine, keeping the DVE free for the recip chain
            for hf in range(2):
                nc.scalar.copy(
                    ct[64 * hf:64 * hf + 64, cl, :], pvt_last[0:64, hf, :])

            # final part 2: all four q-tiles accumulate concurrently
            # (t0/t1 in the freed scores pool, t2 in the pv pool, t3 in
            # pssmall), c4-c6 matmuls first so they overlap the last
            # chunk's normalize chain; the broadcast-normalize runs
            # col-tiled into a single pssmall bank.
            ybs = {}
            for t in range(min(2, ntq)):
                yb = scpool.tile([P, 2, Lq], F32, tag="sc")
                ybs[t] = yb
                final_mms(t, [yb[:, 0, :], yb[:, 1, :]],
                          list(range(FIN_SPLIT, NCH - 1)),
                          first=FIN_SPLIT, last=NCH - 1)
            bcp = pssmall.tile([P, Lq], F32, tag="small")
            for hf in range(2):
                nc.tensor.matmul(
                    out=bcp[64 * hf:64 * hf + 64, :],
                    lhsT=ones[0:1, 64 * hf:64 * hf + 64],
                    rhs=rrowb[0:1, hf, :], start=True, stop=True)
            if ntq > 2:
                yb = pvpool.tile([P, 2, Lq], F32, tag="pv")
                ybs[2] = yb
                final_mms(2, [yb[:, 0, :], yb[:, 1, :]],
                          list(range(FIN_SPLIT, NCH - 1)),
                          first=FIN_SPLIT, last=NCH - 1)
            for hf in range(2):
                sl = ct[64 * hf:64 * hf + 64, cl, :]
                nc.vector.tensor_mul(sl, sl, bcp[64 * hf:64 * hf + 64, :])
            if ntq > 3:
                yb0 = pssmall.tile([P, 512], F32, tag="small")
                yb1 = pssmall.tile([P, 512], F32, tag="small")
                ybs[3] = None
                final_mms(3, [yb0, yb1], list(range(FIN_SPLIT, NCH - 1)),
                          first=FIN_SPLIT, last=NCH - 1)
                yb3 = [yb0, yb1]
            for t in range(ntq):
                yb = ybs[t]
                ytiles = yb3 if yb is None else [yb[:, 0, :], yb[:, 1, :]]
                # fold the part-1 sums in via identity matmuls (PE has
                # slack here), then the chunk-7 stop matmuls; evacuate
                # on the idle scalar engine -- the DVE stays off the
                # tail critical path entirely
                for eh in range(2):
                    nc.tensor.matmul(
                        out=ytiles[eh][:], lhsT=eyes[:],
                        rhs=ysum[:, t, 512 * eh:512 * (eh + 1)],
                        start=False, stop=False, skip_group_check=True)
                final_mms(t, ytiles, [NCH - 1],
                          first=FIN_SPLIT, last=NCH - 1)
                ys = ystage.tile([P, E], BF16, tag="ys")
                with nc.allow_low_precision(reason="bf16 output"):
                    nc.scalar.copy(ys[:, 0:512], ytiles[0][:])
                    nc.scalar.copy(ys[:, 512:1024], ytiles[1][:])
                eng = nc.gpsimd if t % 2 == 0 else nc.sync
                eng.dma_start(out=Y[t * P:(t + 1) * P, :], in_=ys[:])

    nc.compile()
    return nc


def make_core_inputs(Q, K, V, HeadLinear, OutputLiner, QMask, KMask):
    """Host-side sharding/compaction/projection.

    Returns (in_maps, qidxs, ntq, ntk).  qidxs[b] holds the query
    indices the DEVICE computes.  If the max valid-query count is only
    slightly above a 512 multiple (<= 64 over), the device is capped at
    that multiple and the few overflow queries are computed exactly on
    the host during gather (see _host_tail in kernel()).

    The HeadLinear projection is applied on the host (fp32), and V is
    packed as [Lk, H, 65] with a per-key validity column per head.
    """
    bf16 = ml_dtypes.bfloat16
    qm = np.asarray(QMask).astype(bool)
    km = np.asarray(KMask).astype(bool)
    qidxs = [np.nonzero(qm[b])[0] for b in range(B)]
    kidxs = [np.nonzero(km[b])[0] for b in range(B)]
    maxq = max(len(ix) for ix in qidxs)
    qcap = maxq
    if maxq > 512 and maxq % 512 <= 64:
        qcap = (maxq // 512) * 512
    qidxs = [ix[:qcap] for ix in qidxs]
    ntq = max(1, math.ceil(max(len(ix) for ix in qidxs) / P))
    ntk = max(1, math.ceil(max(len(ix) for ix in kidxs) / P))
    Lq, Lk = ntq * P, ntk * P

    hl = np.asarray(HeadLinear, dtype=np.float32)
    ob = np.asarray(OutputLiner, dtype=np.float32).astype(bf16)
    eye = np.eye(P, dtype=np.float32).astype(bf16)

    def proj(x):
        # x [n, E] -> [H*D, n] head-major (chunk c rows = heads 2c,2c+1)
        xh = x.reshape(-1, H, D).transpose(1, 0, 2)  # [H, n, D]
        ph = np.matmul(xh, hl)                       # [H, n, D]
        return ph.transpose(0, 2, 1).reshape(E, -1)

    in_maps = []
    for b in range(B):
        qi, ki = qidxs[b], kidxs[b]
        qc = np.zeros((Lq, E), dtype=np.float32)
        qc[:len(qi)] = np.asarray(Q[b], dtype=np.float32)[qi]
        kc = np.zeros((Lk, E), dtype=np.float32)
        kc[:len(ki)] = np.asarray(K[b], dtype=np.float32)[ki]
        vc = np.zeros((Lk, E), dtype=np.float32)
        vc[:len(ki)] = np.asarray(V[b], dtype=np.float32)[ki]

        vh = np.matmul(vc.reshape(Lk, H, D).transpose(1, 0, 2), hl)  # [H,Lk,D]
        vs = np.zeros((Lk, H, DV), dtype=np.float32)
        vs[:, :, 0:D] = vh.transpose(1, 0, 2)
        vs[:len(ki), :, D] = 1.0

        in_maps.append({
            "QT": np.ascontiguousarray(proj(qc).astype(bf16)),
            "KT": np.ascontiguousarray(proj(kc).astype(bf16)),
            "VS": np.ascontiguousarray(vs.reshape(Lk, H * DV).astype(bf16)),
            "OB": ob, "EYE": eye,
        })
    return in_maps, qidxs, ntq, ntk


_NC_CACHE = {}


def _get_nc(ntq, ntk):
    if (ntq, ntk) not in _NC_CACHE:
        _NC_CACHE[(ntq, ntk)] = build_bass(ntq, ntk)
    return _NC_CACHE[(ntq, ntk)]


def _host_tail(Q, K, V, HeadLinear, OutputLiner, KMask, b, tidx):
    """Exact fp32 attention for a few overflow queries of batch b."""
    hl = np.asarray(HeadLinear, dtype=np.float32)
    ob = np.asarray(OutputLiner, dtype=np.float32)
    ki = np.nonzero(np.asarray(KMask[b]).astype(bool))[0]
    q = np.asarray(Q[b], dtype=np.float32)[tidx]
    kk = np.asarray(K[b], dtype=np.float32)[ki]
    vv = np.asarray(V[b], dtype=np.float32)[ki]
    outs = []
    for h in range(H):
        sl = slice(h * D, (h + 1) * D)
        qh = q[:, sl] @ hl[h]
        kh = kk[:, sl] @ hl[h]
        vh = vv[:, sl] @ hl[h]
        s = (qh @ kh.T) / np.float32(np.sqrt(D))
        s -= s.max(axis=1, keepdims=True)
        p = np.exp(s)
        p /= p.sum(axis=1, keepdims=True)
        outs.append(p @ vh)
    return np.concatenate(outs, axis=1) @ ob


def kernel(Q, K, V, HeadLinear, OutputLiner, QMask, KMask):
    from concourse.bass_utils import run_bass_kernel_spmd

    in_maps, qidxs, ntq, ntk = make_core_inputs(
        Q, K, V, HeadLinear, OutputLiner, QMask, KMask)
    nc = _get_nc(ntq, ntk)
    res = run_bass_kernel_spmd(nc, in_maps, list(range(B)))
    out = np.zeros((B, L, E), dtype=np.float32)
    qm = np.asarray(QMask).astype(bool)
    for b in range(B):
        yc = np.asarray(res.results[b]["Y"]).astype(np.float32)
        out[b][qidxs[b]] = yc[:len(qidxs[b])]
        full = np.nonzero(qm[b])[0]
        tidx = full[len(qidxs[b]):]
        if len(tidx):
            out[b][tidx] = _host_tail(
                Q, K, V, HeadLinear, OutputLiner, KMask, b, tidx)
    return out
